# revision 16
# baseline (speedup 1.0000x reference)
"""Trainium2 Bass kernel for nn_DCTLayer: per-8x8-block 2D DCT-like transform.

Math: reference computes, per 8x8 block X of the 256x256 image,
    out_block[y, v] = sum_x A[v, x] * X[x, y],   where A = D @ D
(D = 8x8 DCT basis). out_block = (A @ X)^T.

Kernel strategy (per core, pure data parallel over batch):
  - Load 128 consecutive image rows into SBUF naturally: partition = (G, x)
    [p = 8G+x], free = (r, c) [r = row-half, c = column]. Contiguous 1KB-run
    DMA.
  - Matmul per (r, h): stationary = data with free AP ordered (y, j)
    [c = 128h + 8j + y], moving = constant BD2[8g+x, 16v+g] = A[v,x].
    PSUM out: partition (y, j) [p = 16y+j], written STRIDED so the h-half of
    PSUM has free layout f = 32v + 16r + g.
  - DVE stream-transpose (32x32 blocks) per h-half: swaps partition-lo5
    (y0, j) with free-lo5 (r, g), yielding partition p' = 32Y + 16r + g
    [Y = y>>1], free (v, y0, j).  This is the cross-partition shuffle that
    makes output rows contiguous per partition.
  - Reorder copy (v, y0, j) -> st2 layout (y0, h, j, v): now partition p'
    holds output rows 8q + 2Y + y0 (q = p' mod 32) as full 1KB spans.
  - Store per Y-group: 3-dim AP [q(32 partitions), y0, 1KB row run];
    row = 8q + 2Y + y0 is affine in the partition index.  4 store DMAs per
    image with 1KB contiguous runs (full DMA bandwidth) vs the old 16 DMAs
    of 32B runs.
"""

import sys

sys.path.insert(0, "/opt/trn_rl_repo")

from contextlib import ExitStack

import numpy as np

import concourse.bass as bass  # noqa: F401
import concourse.tile as tile
from concourse import bacc, mybir
from concourse.bass_utils import run_bass_kernel_spmd

P = 8
H = W = 256
B, C = 16, 64
NCORES = 8
BPC = B // NCORES  # batches per core
IMGS = BPC * C  # images (b,c planes) per core
ROWS = IMGS * H  # dram rows per core

TRACE = False
LAST_RESULTS = None

_nc_cache = None


def _ensure_ntff_hook():
    """The agent image's antenv lacks axon_hooks; synthesize it so
    run_bass_kernel_spmd(trace=True) can capture NTFF profiles."""
    import types

    if "antenv.axon_hooks" in sys.modules:
        return
    try:
        sys.path.insert(0, "/root/.axon_site/trn_agent_boot")
        from trn_boot import _ntff_profile_via_ctypes

        hook = _ntff_profile_via_ctypes("/opt/axon/libaxon_pjrt.so")
    except Exception:
        hook = None
    mod = types.ModuleType("antenv.axon_hooks")
    mod._hook = hook
    mod.get_axon_ntff_profile_hook = lambda: mod._hook
    mod.set_axon_ntff_profile_hook = lambda h: setattr(mod, "_hook", h)
    sys.modules["antenv.axon_hooks"] = mod


def _dct_kernel(tc, o, x, bd):
    nc = tc.nc
    with ExitStack() as ctx:
        xpool = ctx.enter_context(tc.tile_pool(name="xin", bufs=6))
        x2pool = ctx.enter_context(tc.tile_pool(name="xperm", bufs=4))
        zpool = ctx.enter_context(tc.tile_pool(name="zint", bufs=4))
        spool = ctx.enter_context(tc.tile_pool(name="stt", bufs=4))
        s2pool = ctx.enter_context(tc.tile_pool(name="st2", bufs=4))
        cpool = ctx.enter_context(tc.tile_pool(name="const", bufs=1))
        ppool = ctx.enter_context(tc.tile_pool(name="ps", bufs=4, space="PSUM"))

        bdt = cpool.tile([128, 128], mybir.dt.float32)
        nc.sync.dma_start(bdt[:], bd[:])

        for img in range(IMGS):
            # ---- load image (256x256) as [p=(G,x), (r, c)]; 1KB runs ----
            xt = xpool.tile([128, 2 * W], mybir.dt.float32)
            src = x[img * H : (img + 1) * H, :].rearrange("(r p) c -> p r c", p=128)
            dst = xt[:].rearrange("p (r c) -> p r c", c=W)
            nc.sync.dma_start(dst, src)

            # ---- pre-permute cols to y-major so matmul APs stay 1-D ----
            # xt  free: (r, h, j, y)   ->   xt2 free: (r, h, y, j)
            xt2 = x2pool.tile([128, 2 * W], mybir.dt.float32)
            for r in range(2):
                cin = xt[:, r * W : (r + 1) * W].rearrange(
                    "p (h j y) -> p h y j", h=2, j=16, y=8
                )
                cout = xt2[:, r * W : (r + 1) * W].rearrange(
                    "p (h y j) -> p h y j", h=2, y=8, j=16
                )
                nc.vector.tensor_copy(cout, cin)

            # ---- 4 plain matmuls: psum chunk at 256h+128r = [p=(y,j), (v,g)] ----
            ps = ppool.tile([128, 512], mybir.dt.float32)
            for r in range(2):
                for h in range(2):
                    stat = xt2[:, r * W + h * 128 : r * W + (h + 1) * 128]
                    pout = ps[:, h * 256 + r * 128 : h * 256 + r * 128 + 128]
                    nc.tensor.matmul(pout, stat, bdt[:], start=True, stop=True)

            # ---- interleave copy per h: psum (r, v, g) -> zt (v, r, g) ----
            zt = zpool.tile([128, 512], mybir.dt.float32)
            for h in range(2):
                cin = ps[:, h * 256 : (h + 1) * 256].rearrange(
                    "p (r v g) -> p r v g", r=2, v=8, g=16
                )
                cout = zt[:, h * 256 : (h + 1) * 256].rearrange(
                    "p (v r g) -> p r v g", v=8, r=2, g=16
                )
                nc.scalar.mul(cout, cin, 1.0)

            # ---- DVE stream transpose, both halves in one instr ----
            # in:  partition (Y, y0, j), free (h, v, r, g)
            # out: partition (Y, r, g),  free (h, v, y0, j)
            st = spool.tile([128, 512], mybir.dt.float32)
            nc.vector.transpose(st[:], zt[:])

            # ---- reorder copy: free (v, y0, j) -> st2 (y0, h, j, v) ----
            st2 = s2pool.tile([128, 512], mybir.dt.float32)
            for h in range(2):
                for y0 in range(2):
                    if h == 0:
                        eng = nc.scalar
                    else:
                        eng = nc.gpsimd if y0 == 0 else nc.vector
                    cin = st[:, h * 256 : (h + 1) * 256].rearrange(
                        "p (v yz j) -> p yz v j", v=8, yz=2, j=16
                    )[:, y0 : y0 + 1, :, :]
                    base2 = y0 * 256 + h * 128
                    cout = st2[:, base2 : base2 + 128].rearrange(
                        "p (o j v) -> p o v j", o=1, j=16, v=8
                    )
                    if eng is nc.scalar:
                        eng.mul(cout, cin, 1.0)
                    else:
                        eng.tensor_copy(cout, cin)

            # ---- one store per image: dram row' = 2*p' + y0 (host unshuffles)
            ssrc = st2[:].rearrange("p (y0 c) -> p y0 c", y0=2, c=256)
            sdst = o[img * H : (img + 1) * H, :].rearrange(
                "(q y0) c -> q y0 c", y0=2
            )
            nc.gpsimd.dma_start(sdst, ssrc)


def _build_nc():
    nc = bacc.Bacc(
        "TRN2", target_bir_lowering=False, debug=False, num_devices=NCORES
    )
    x_ap = nc.dram_tensor("x", [ROWS, W], mybir.dt.float32, kind="ExternalInput").ap()
    bd_ap = nc.dram_tensor(
        "bd", [128, 128], mybir.dt.float32, kind="ExternalInput"
    ).ap()
    o_ap = nc.dram_tensor("o", [ROWS, W], mybir.dt.float32, kind="ExternalOutput").ap()
    with tile.TileContext(nc) as tc:
        _dct_kernel(tc, o_ap, x_ap, bd_ap)
    nc.compile()
    return nc


def _make_bd(dct_basis: np.ndarray) -> np.ndarray:
    a = dct_basis.astype(np.float64) @ dct_basis.astype(np.float64)
    a = a.astype(np.float32)
    bd = np.zeros((128, 128), dtype=np.float32)
    for g in range(16):
        for x in range(P):
            for v in range(P):
                bd[8 * g + x, 16 * v + g] = a[v, x]
    return bd


def kernel(x: np.ndarray, dct_basis: np.ndarray) -> np.ndarray:
    global _nc_cache, LAST_RESULTS
    x = np.asarray(x, dtype=np.float32)
    dct_basis = np.asarray(dct_basis, dtype=np.float32)
    assert x.shape == (B, C, H, W)

    if _nc_cache is None:
        _nc_cache = _build_nc()
    nc = _nc_cache

    bd = _make_bd(dct_basis)
    in_maps = []
    for i in range(NCORES):
        xs = np.ascontiguousarray(x[i * BPC : (i + 1) * BPC]).reshape(ROWS, W)
        in_maps.append({"x": xs, "bd": bd})

    if TRACE:
        _ensure_ntff_hook()
    try:
        res = run_bass_kernel_spmd(
            nc, in_maps, core_ids=list(range(NCORES)), trace=TRACE
        )
    except ModuleNotFoundError:
        res = run_bass_kernel_spmd(
            nc, in_maps, core_ids=list(range(NCORES)), trace=False
        )
    LAST_RESULTS = res

    out = np.empty((B, C, H, W), dtype=np.float32)
    for i in range(NCORES):
        # device rows are (img, Y, r, g, y0); true rows are (r, g, Y, y0)
        oc = res.results[i]["o"].reshape(IMGS, 4, 2, 16, 2, W)
        oc = oc.transpose(0, 2, 3, 1, 4, 5).reshape(BPC, C, H, W)
        out[i * BPC : (i + 1) * BPC] = oc
    return out
